# revision 49
# baseline (speedup 1.0000x reference)
"""MoE gate (router) kernel for Trainium2, 8 NeuronCores, data-parallel.

reference: logits = x @ W_g  ([16384,2048] @ [2048,64]); scores = softmax(logits);
           return top-6 (indices, scores).

Strategy
--------
Data-parallel over tokens: each of the 8 cores handles 2048 tokens. The
contraction dim K=2048 must live on SBUF partitions, so the host pre-arranges
the input. fp32 matmul on PE is ~4x slow with un-hidden fp32 LDWEIGHTS, so x
and W are split on the host into fp16 hi/lo pairs (lo scaled by 2^11 to stay
in fp16 normal range):

    x = xh + 2^-11 * xl',   W = Wh + 2^-11 * Wl'     (all fp16, exact split)
    logits = xh@Wh + 2^-11 * (xh@Wl' + xl'@Wh)  (+ 2^-22 xl'@Wl', dropped)

All fp16 products are exact in fp32 PSUM accumulation -> ~2^-22 relative
logit error, i.e. fp32-class. PE runs at full rate with fast weight load.

DMA design (the kernel is HBM-bandwidth bound):
  - Host packs x as [128(partition), nt(block), 16(chunk), 2(hi/lo), 128(tok)]
    so each 128-token block is ONE dma_start with a single contiguous 8KiB
    segment per partition: maximal packet efficiency, one completion
    semaphore per block, issue cost ~2.8ns/KB of descriptor generation.
  - Blocks alternate between the sync HWDGE ring and the gpsimd SWDGE ring;
    each ring's descriptor generation (~380GB/s) alone barely covers HBM
    bandwidth, two together stay comfortably ahead, and arrivals track
    consumption order with ~1 block of skew.
  - PE consumes a block (32 matmuls, ~3us) in about the time one block
    streams, so compute trails the stream by ~1 block into a short tail.

Per 128-token block:
  - group A = 16 matmuls xh_c @ [Wh|Wl']_c (N=128) -> PSUM [128,128]
  - group B = 16 matmuls xl'_c @ Wh_c (N=64)      -> PSUM [128,64]
  - fold (3 DVE ops): lg = A[:,0:64] + 2^-11*(A[:,64:128] + B)
  - softmax+top6 without max-subtraction (|logits| < ~6): erow=exp(lg) (ACT),
    sume (DVE reduce), max8/find_index8 on raw logits, scores=exp(v6)/sume.
  - Outputs staged in SBUF [128, nt, 6] (partition-major; host reorders),
    one DMA per output at the end.
"""

import os
import sys

import numpy as np

for _p in ("/opt/trn_rl_repo", "/root/.axon_site/_ro/trn_rl_repo"):
    if os.path.isdir(_p) and _p not in sys.path:
        sys.path.insert(0, _p)

import concourse.bass as bass
import concourse.mybir as mybir
from concourse import bacc, bass_utils
from concourse.tile import TileContext

N_CORES = 8
T_FULL = 16384
K = 2048
E = 64
TOPK = 6
P = 128
KC = K // P  # 16 contraction chunks
LO_SCALE = 2048.0  # 2^11

_NC_CACHE: dict[int, "bass.Bass"] = {}
LAST_RESULT = None  # BassKernelResults of the most recent kernel() call


def build_nc(t_shard: int = T_FULL // N_CORES) -> "bass.Bass":
    f16 = mybir.dt.float16
    f32 = mybir.dt.float32
    i32 = mybir.dt.int32
    u32 = mybir.dt.uint32
    EXP = mybir.ActivationFunctionType.Exp

    assert t_shard % P == 0
    nt = t_shard // P  # number of 128-token blocks
    S = KC * 2 * P  # elements per (partition, block) = 4096 (8KiB fp16)

    nc = bacc.Bacc()
    xhl = nc.dram_tensor("xhl", [P, nt * S], f16, kind="ExternalInput")
    Ws = nc.dram_tensor("Ws", [K, 2 * E], f16, kind="ExternalInput")
    # outputs in partition-major layout [P, nt, TOPK]; host reorders.
    idx_o = nc.dram_tensor("idx", [P, nt, TOPK], i32, kind="ExternalOutput")
    val_o = nc.dram_tensor("val", [P, nt, TOPK], f32, kind="ExternalOutput")

    with TileContext(nc) as tc:
        with (
            tc.tile_pool(name="singles", bufs=1) as singles,
            tc.tile_pool(name="xpool", bufs=6) as xpool,
            tc.tile_pool(name="small", bufs=4) as small,
            tc.tile_pool(name="psum", bufs=4, space="PSUM") as psum_pool,
        ):
            W_sb = singles.tile([P, KC, 2 * E], f16)
            nc.scalar.dma_start(
                out=W_sb, in_=Ws[:].rearrange("(c p) e -> p c e", p=P)
            )
            vstage = singles.tile([P, nt, TOPK], f32)
            istage = singles.tile([P, nt, TOPK], i32)

            for b in range(nt):
                xb = xpool.tile([P, KC, 2, P], f16, tag="xb")
                eng = (nc.sync, nc.gpsimd)[b % 2]
                eng.dma_start(
                    out=xb,
                    in_=xhl[:, b * S : (b + 1) * S].rearrange(
                        "p (c h t) -> p c h t", c=KC, h=2
                    ),
                )
                psA = psum_pool.tile([P, 2 * E], f32, tag="psA")
                psB = psum_pool.tile([P, E], f32, tag="psB")
                for c in range(KC):
                    nc.tensor.matmul(
                        psA,
                        xb[:, c, 0],
                        W_sb[:, c],
                        start=(c == 0),
                        stop=(c == KC - 1),
                    )
                for c in range(KC):
                    nc.tensor.matmul(
                        psB,
                        xb[:, c, 1],
                        W_sb[:, c, :E],
                        start=(c == 0),
                        stop=(c == KC - 1),
                    )
                # fold: lg = A[:, :64] + 2^-11 * (A[:, 64:] + B)
                t1 = small.tile([P, E], f32, tag="t1")
                nc.vector.tensor_copy(t1, psB)
                t2 = small.tile([P, E], f32, tag="t2")
                nc.vector.tensor_add(t2, psA[:, E:], t1)
                lg = small.tile([P, E], f32, tag="lg")
                nc.vector.scalar_tensor_tensor(
                    out=lg,
                    in0=t2,
                    scalar=1.0 / LO_SCALE,
                    in1=psA[:, :E],
                    op0=mybir.AluOpType.mult,
                    op1=mybir.AluOpType.add,
                )
                # softmax + top-6 (no max subtraction; |logits| < ~6)
                erow = small.tile([P, E], f32, tag="erow")
                sume = small.tile([P, 1], f32, tag="sume")
                nc.scalar.activation(erow, lg, EXP)
                nc.vector.tensor_reduce(
                    sume, erow, axis=mybir.AxisListType.X, op=mybir.AluOpType.add
                )
                v8 = small.tile([P, 8], f32, tag="v8")
                nc.vector.max(out=v8, in_=lg)
                i8 = small.tile([P, 8], u32, tag="i8")
                nc.vector.max_index(out=i8, in_max=v8, in_values=lg)
                rec = small.tile([P, 1], f32, tag="rec")
                nc.vector.reciprocal(rec, sume)
                ev = small.tile([P, TOPK], f32, tag="ev")
                nc.scalar.activation(ev, v8[:, :TOPK], EXP)
                nc.vector.tensor_scalar_mul(vstage[:, b], ev, rec)
                nc.vector.tensor_copy(istage[:, b], i8[:, :TOPK])

            nc.sync.dma_start(out=idx_o[:], in_=istage)
            nc.scalar.dma_start(out=val_o[:], in_=vstage)
    if not nc.is_finalized():
        nc.finalize()
    return nc


def _get_nc(t_shard: int) -> "bass.Bass":
    if t_shard not in _NC_CACHE:
        _NC_CACHE[t_shard] = build_nc(t_shard)
    return _NC_CACHE[t_shard]


def _split_hi_lo(a: np.ndarray) -> tuple[np.ndarray, np.ndarray]:
    hi = a.astype(np.float16)
    lo = ((a - hi.astype(np.float32)) * np.float32(LO_SCALE)).astype(np.float16)
    return hi, lo


def pack_core_input(xh: np.ndarray, xl: np.ndarray) -> np.ndarray:
    """[t_shard, K] hi/lo -> [P, nt*KC*2*P] packed for the kernel."""
    t_shard = xh.shape[0]
    nt = t_shard // P
    # [t, (c p)] -> [p, b(t-tile), c, h, t-in-tile]
    xp = np.empty((P, nt, KC, 2, P), np.float16)
    hiT = xh.T.reshape(KC, P, nt, P)  # [c, p, b, t]
    loT = xl.T.reshape(KC, P, nt, P)
    xp[:, :, :, 0, :] = hiT.transpose(1, 2, 0, 3)
    xp[:, :, :, 1, :] = loT.transpose(1, 2, 0, 3)
    return np.ascontiguousarray(xp.reshape(P, nt * KC * 2 * P))


def kernel(x: np.ndarray, W_g: np.ndarray, **run_kwargs):
    global LAST_RESULT
    x = np.asarray(x, dtype=np.float32)
    W = np.asarray(W_g, dtype=np.float32)
    t_shard = x.shape[0] // N_CORES
    nc = _get_nc(t_shard)

    xh, xl = _split_hi_lo(x)
    Wh, Wl = _split_hi_lo(W)
    Ws = np.ascontiguousarray(np.concatenate([Wh, Wl], axis=1))
    in_maps = [
        {
            "xhl": pack_core_input(
                xh[c * t_shard : (c + 1) * t_shard],
                xl[c * t_shard : (c + 1) * t_shard],
            ),
            "Ws": Ws,
        }
        for c in range(N_CORES)
    ]
    res = bass_utils.run_bass_kernel_spmd(
        nc, in_maps, core_ids=list(range(N_CORES)), **run_kwargs
    )
    LAST_RESULT = res
    # device layout is [P, nt, TOPK]; token t = tile*P + p -> [t_shard, TOPK]
    idx = np.concatenate(
        [np.moveaxis(r["idx"], 0, 1).reshape(t_shard, TOPK) for r in res.results],
        axis=0,
    ).astype(np.int32)
    val = np.concatenate(
        [np.moveaxis(r["val"], 0, 1).reshape(t_shard, TOPK) for r in res.results],
        axis=0,
    ).astype(np.float32)
    return idx, val


# revision 52
# speedup vs baseline: 1.0703x; 1.0703x over previous
"""MoE gate (router) kernel for Trainium2, 8 NeuronCores, data-parallel.

reference: logits = x @ W_g  ([16384,2048] @ [2048,64]); scores = softmax(logits);
           return top-6 (indices, scores).

Strategy
--------
Data-parallel over tokens: each of the 8 cores handles 2048 tokens. The
contraction dim K=2048 must live on SBUF partitions, so the host pre-arranges
the input. fp32 matmul on PE is ~4x slow with un-hidden fp32 LDWEIGHTS, so x
and W are split on the host into fp16 hi/lo pairs (lo scaled by 2^11 to stay
in fp16 normal range):

    x = xh + 2^-11 * xl',   W = Wh + 2^-11 * Wl'     (all fp16, exact split)
    logits = xh@Wh + 2^-11 * (xh@Wl' + xl'@Wh)  (+ 2^-22 xl'@Wl', dropped)

All fp16 products are exact in fp32 PSUM accumulation -> ~2^-22 relative
logit error, i.e. fp32-class. PE runs at full rate with fast weight load.

DMA design (the kernel is HBM-bandwidth bound):
  - Host packs x as [128(partition), nt(block), 16(chunk), 2(hi/lo), 128(tok)]
    so each 128-token block is ONE dma_start with a single contiguous 8KiB
    segment per partition: maximal packet efficiency, one completion
    semaphore per block, issue cost ~2.8ns/KB of descriptor generation.
  - Blocks alternate between the sync HWDGE ring and the gpsimd SWDGE ring;
    each ring's descriptor generation (~380GB/s) alone barely covers HBM
    bandwidth, two together stay comfortably ahead, and arrivals track
    consumption order with ~1 block of skew.
  - PE consumes a block (32 matmuls, ~3us) in about the time one block
    streams, so compute trails the stream by ~1 block into a short tail.

Per 128-token block:
  - group A = 16 matmuls xh_c @ [Wh|Wl']_c (N=128) -> PSUM [128,128]
  - group B = 16 matmuls xl'_c @ Wh_c (N=64)      -> PSUM [128,64]
  - fold (3 DVE ops): lg = A[:,0:64] + 2^-11*(A[:,64:128] + B)
  - softmax+top6 without max-subtraction (|logits| < ~6): erow=exp(lg) (ACT),
    sume (DVE reduce), max8/find_index8 on raw logits, scores=exp(v6)/sume.
  - Outputs staged in SBUF [128, nt, 6] (partition-major; host reorders),
    one DMA per output at the end.
"""

import os
import sys

import numpy as np

for _p in ("/opt/trn_rl_repo", "/root/.axon_site/_ro/trn_rl_repo"):
    if os.path.isdir(_p) and _p not in sys.path:
        sys.path.insert(0, _p)

import concourse.bass as bass
import concourse.mybir as mybir
from concourse import bacc, bass_utils
from concourse.tile import TileContext

N_CORES = 8
T_FULL = 16384
K = 2048
E = 64
TOPK = 6
P = 128
KC = K // P  # 16 contraction chunks
LO_SCALE = 2048.0  # 2^11

_NC_CACHE: dict[int, "bass.Bass"] = {}
LAST_RESULT = None  # BassKernelResults of the most recent kernel() call


def build_nc(t_shard: int = T_FULL // N_CORES) -> "bass.Bass":
    f16 = mybir.dt.float16
    f32 = mybir.dt.float32
    i32 = mybir.dt.int32
    u32 = mybir.dt.uint32
    EXP = mybir.ActivationFunctionType.Exp

    assert t_shard % P == 0
    nt = t_shard // P  # number of 128-token blocks
    S = KC * 2 * P  # elements per (partition, block) = 4096 (8KiB fp16)

    nc = bacc.Bacc()
    xhl = nc.dram_tensor("xhl", [P, nt * S], f16, kind="ExternalInput")
    # W packed partition-major too: one 4KiB segment per partition (256B
    # packets would round-robin against the 8KiB x packets and crawl).
    Ws = nc.dram_tensor("Ws", [P, KC * 2 * E], f16, kind="ExternalInput")
    # outputs in partition-major layout [P, nt, TOPK]; host reorders.
    idx_o = nc.dram_tensor("idx", [P, nt, TOPK], i32, kind="ExternalOutput")
    val_o = nc.dram_tensor("val", [P, nt, TOPK], f32, kind="ExternalOutput")

    with TileContext(nc) as tc:
        with (
            tc.tile_pool(name="singles", bufs=1) as singles,
            tc.tile_pool(name="xpool", bufs=6) as xpool,
            tc.tile_pool(name="small", bufs=4) as small,
            tc.tile_pool(name="psum", bufs=4, space="PSUM") as psum_pool,
        ):
            W_sb = singles.tile([P, KC, 2 * E], f16)
            nc.sync.dma_start(
                out=W_sb, in_=Ws[:].rearrange("p (c e) -> p c e", c=KC)
            )
            vstage = singles.tile([P, nt, TOPK], f32)
            istage = singles.tile([P, nt, TOPK], i32)

            for b in range(nt):
                xb = xpool.tile([P, KC, 2, P], f16, tag="xb")
                eng = (nc.sync, nc.gpsimd)[b % 2]
                eng.dma_start(
                    out=xb,
                    in_=xhl[:, b * S : (b + 1) * S].rearrange(
                        "p (c h t) -> p c h t", c=KC, h=2
                    ),
                )
                psA = psum_pool.tile([P, 2 * E], f32, tag="psA")
                psB = psum_pool.tile([P, E], f32, tag="psB")
                for c in range(KC):
                    nc.tensor.matmul(
                        psA,
                        xb[:, c, 0],
                        W_sb[:, c],
                        start=(c == 0),
                        stop=(c == KC - 1),
                    )
                for c in range(KC):
                    nc.tensor.matmul(
                        psB,
                        xb[:, c, 1],
                        W_sb[:, c, :E],
                        start=(c == 0),
                        stop=(c == KC - 1),
                    )
                # fold: lg = A[:, :64] + 2^-11 * (A[:, 64:] + B)
                t1 = small.tile([P, E], f32, tag="t1")
                nc.vector.tensor_copy(t1, psB)
                t2 = small.tile([P, E], f32, tag="t2")
                nc.vector.tensor_add(t2, psA[:, E:], t1)
                lg = small.tile([P, E], f32, tag="lg")
                nc.vector.scalar_tensor_tensor(
                    out=lg,
                    in0=t2,
                    scalar=1.0 / LO_SCALE,
                    in1=psA[:, :E],
                    op0=mybir.AluOpType.mult,
                    op1=mybir.AluOpType.add,
                )
                # softmax + top-6 (no max subtraction; |logits| < ~6)
                erow = small.tile([P, E], f32, tag="erow")
                sume = small.tile([P, 1], f32, tag="sume")
                nc.scalar.activation(erow, lg, EXP)
                nc.vector.tensor_reduce(
                    sume, erow, axis=mybir.AxisListType.X, op=mybir.AluOpType.add
                )
                v8 = small.tile([P, 8], f32, tag="v8")
                nc.vector.max(out=v8, in_=lg)
                i8 = small.tile([P, 8], u32, tag="i8")
                nc.vector.max_index(out=i8, in_max=v8, in_values=lg)
                rec = small.tile([P, 1], f32, tag="rec")
                nc.vector.reciprocal(rec, sume)
                ev = small.tile([P, TOPK], f32, tag="ev")
                nc.scalar.activation(ev, v8[:, :TOPK], EXP)
                nc.vector.tensor_scalar_mul(vstage[:, b], ev, rec)
                nc.vector.tensor_copy(istage[:, b], i8[:, :TOPK])

            nc.sync.dma_start(out=idx_o[:], in_=istage)
            nc.scalar.dma_start(out=val_o[:], in_=vstage)
    if not nc.is_finalized():
        nc.finalize()
    return nc


def _get_nc(t_shard: int) -> "bass.Bass":
    if t_shard not in _NC_CACHE:
        _NC_CACHE[t_shard] = build_nc(t_shard)
    return _NC_CACHE[t_shard]


def _split_hi_lo(a: np.ndarray) -> tuple[np.ndarray, np.ndarray]:
    hi = a.astype(np.float16)
    lo = ((a - hi.astype(np.float32)) * np.float32(LO_SCALE)).astype(np.float16)
    return hi, lo


def pack_core_input(xh: np.ndarray, xl: np.ndarray) -> np.ndarray:
    """[t_shard, K] hi/lo -> [P, nt*KC*2*P] packed for the kernel."""
    t_shard = xh.shape[0]
    nt = t_shard // P
    # [t, (c p)] -> [p, b(t-tile), c, h, t-in-tile]
    xp = np.empty((P, nt, KC, 2, P), np.float16)
    hiT = xh.T.reshape(KC, P, nt, P)  # [c, p, b, t]
    loT = xl.T.reshape(KC, P, nt, P)
    xp[:, :, :, 0, :] = hiT.transpose(1, 2, 0, 3)
    xp[:, :, :, 1, :] = loT.transpose(1, 2, 0, 3)
    return np.ascontiguousarray(xp.reshape(P, nt * KC * 2 * P))


def kernel(x: np.ndarray, W_g: np.ndarray, **run_kwargs):
    global LAST_RESULT
    x = np.asarray(x, dtype=np.float32)
    W = np.asarray(W_g, dtype=np.float32)
    t_shard = x.shape[0] // N_CORES
    nc = _get_nc(t_shard)

    xh, xl = _split_hi_lo(x)
    Wh, Wl = _split_hi_lo(W)
    Wstk = np.concatenate([Wh, Wl], axis=1)  # [K, 2E]
    # [(c p), e] -> [p, (c e)]
    Ws = np.ascontiguousarray(
        Wstk.reshape(KC, P, 2 * E).transpose(1, 0, 2).reshape(P, KC * 2 * E)
    )
    in_maps = [
        {
            "xhl": pack_core_input(
                xh[c * t_shard : (c + 1) * t_shard],
                xl[c * t_shard : (c + 1) * t_shard],
            ),
            "Ws": Ws,
        }
        for c in range(N_CORES)
    ]
    res = bass_utils.run_bass_kernel_spmd(
        nc, in_maps, core_ids=list(range(N_CORES)), **run_kwargs
    )
    LAST_RESULT = res
    # device layout is [P, nt, TOPK]; token t = tile*P + p -> [t_shard, TOPK]
    idx = np.concatenate(
        [np.moveaxis(r["idx"], 0, 1).reshape(t_shard, TOPK) for r in res.results],
        axis=0,
    ).astype(np.int32)
    val = np.concatenate(
        [np.moveaxis(r["val"], 0, 1).reshape(t_shard, TOPK) for r in res.results],
        axis=0,
    ).astype(np.float32)
    return idx, val


# revision 53
# speedup vs baseline: 1.0936x; 1.0218x over previous
"""MoE gate (router) kernel for Trainium2, 8 NeuronCores, data-parallel.

reference: logits = x @ W_g  ([16384,2048] @ [2048,64]); scores = softmax(logits);
           return top-6 (indices, scores).

Strategy
--------
Data-parallel over tokens: each of the 8 cores handles 2048 tokens. The
contraction dim K=2048 must live on SBUF partitions, so the host pre-arranges
the input. fp32 matmul on PE is ~4x slow with un-hidden fp32 LDWEIGHTS, so x
and W are split on the host into fp16 hi/lo pairs (lo scaled by 2^11 to stay
in fp16 normal range):

    x = xh + 2^-11 * xl',   W = Wh + 2^-11 * Wl'     (all fp16, exact split)
    logits = xh@Wh + 2^-11 * (xh@Wl' + xl'@Wh)  (+ 2^-22 xl'@Wl', dropped)

All fp16 products are exact in fp32 PSUM accumulation -> ~2^-22 relative
logit error, i.e. fp32-class. PE runs at full rate with fast weight load.

DMA design (the kernel is HBM-bandwidth bound):
  - Host packs x as [128(partition), nt(block), 16(chunk), 2(hi/lo), 128(tok)]
    so each 128-token block is ONE dma_start with a single contiguous 8KiB
    segment per partition: maximal packet efficiency, one completion
    semaphore per block, issue cost ~2.8ns/KB of descriptor generation.
  - Blocks alternate between the sync HWDGE ring and the gpsimd SWDGE ring;
    each ring's descriptor generation (~380GB/s) alone barely covers HBM
    bandwidth, two together stay comfortably ahead, and arrivals track
    consumption order with ~1 block of skew.
  - PE consumes a block (32 matmuls, ~3us) in about the time one block
    streams, so compute trails the stream by ~1 block into a short tail.

Per 128-token block:
  - group A = 16 matmuls xh_c @ [Wh|Wl']_c (N=128) -> PSUM [128,128]
  - group B = 16 matmuls xl'_c @ Wh_c (N=64)      -> PSUM [128,64]
  - fold (3 DVE ops): lg = A[:,0:64] + 2^-11*(A[:,64:128] + B)
  - softmax+top6 without max-subtraction (|logits| < ~6): erow=exp(lg) (ACT),
    sume (DVE reduce), max8/find_index8 on raw logits, scores=exp(v6)/sume.
  - Outputs staged in SBUF [128, nt, 6] (partition-major; host reorders),
    one DMA per output at the end.
"""

import os
import sys

import numpy as np

for _p in ("/opt/trn_rl_repo", "/root/.axon_site/_ro/trn_rl_repo"):
    if os.path.isdir(_p) and _p not in sys.path:
        sys.path.insert(0, _p)

import concourse.bass as bass
import concourse.mybir as mybir
from concourse import bacc, bass_utils
from concourse.tile import TileContext

N_CORES = 8
T_FULL = 16384
K = 2048
E = 64
TOPK = 6
P = 128
KC = K // P  # 16 contraction chunks
LO_SCALE = 2048.0  # 2^11

_NC_CACHE: dict[int, "bass.Bass"] = {}
LAST_RESULT = None  # BassKernelResults of the most recent kernel() call


def build_nc(t_shard: int = T_FULL // N_CORES) -> "bass.Bass":
    f16 = mybir.dt.float16
    f32 = mybir.dt.float32
    i32 = mybir.dt.int32
    u32 = mybir.dt.uint32
    EXP = mybir.ActivationFunctionType.Exp

    assert t_shard % P == 0
    nt = t_shard // P  # number of 128-token blocks
    S = KC * 2 * P  # elements per (partition, block) = 4096 (8KiB fp16)

    nc = bacc.Bacc()
    xhl = nc.dram_tensor("xhl", [P, nt * S], f16, kind="ExternalInput")
    # W packed partition-major too: one 4KiB segment per partition (256B
    # packets would round-robin against the 8KiB x packets and crawl).
    Ws = nc.dram_tensor("Ws", [P, KC * 2 * E], f16, kind="ExternalInput")
    # outputs in partition-major layout [P, nt, TOPK]; host reorders.
    idx_o = nc.dram_tensor("idx", [P, nt, TOPK], i32, kind="ExternalOutput")
    val_o = nc.dram_tensor("val", [P, nt, TOPK], f32, kind="ExternalOutput")

    with TileContext(nc) as tc:
        with (
            tc.tile_pool(name="singles", bufs=1) as singles,
            tc.tile_pool(name="xpool", bufs=10) as xpool,
            tc.tile_pool(name="small", bufs=4) as small,
            tc.tile_pool(name="psum", bufs=4, space="PSUM") as psum_pool,
        ):
            W_sb = singles.tile([P, KC, 2 * E], f16)
            nc.sync.dma_start(
                out=W_sb, in_=Ws[:].rearrange("p (c e) -> p c e", c=KC)
            )
            vstage = singles.tile([P, nt, TOPK], f32)
            istage = singles.tile([P, nt, TOPK], i32)

            for b in range(nt):
                xb = xpool.tile([P, KC, 2, P], f16, tag="xb")
                eng = (nc.sync, nc.gpsimd)[b % 2]
                eng.dma_start(
                    out=xb,
                    in_=xhl[:, b * S : (b + 1) * S].rearrange(
                        "p (c h t) -> p c h t", c=KC, h=2
                    ),
                )
                psA = psum_pool.tile([P, 2 * E], f32, tag="psA")
                psB = psum_pool.tile([P, E], f32, tag="psB")
                for c in range(KC):
                    nc.tensor.matmul(
                        psA,
                        xb[:, c, 0],
                        W_sb[:, c],
                        start=(c == 0),
                        stop=(c == KC - 1),
                    )
                for c in range(KC):
                    nc.tensor.matmul(
                        psB,
                        xb[:, c, 1],
                        W_sb[:, c, :E],
                        start=(c == 0),
                        stop=(c == KC - 1),
                    )
                # fold: lg = A[:, :64] + 2^-11 * (A[:, 64:] + B)
                t1 = small.tile([P, E], f32, tag="t1")
                nc.vector.tensor_copy(t1, psB)
                t2 = small.tile([P, E], f32, tag="t2")
                nc.vector.tensor_add(t2, psA[:, E:], t1)
                lg = small.tile([P, E], f32, tag="lg")
                nc.vector.scalar_tensor_tensor(
                    out=lg,
                    in0=t2,
                    scalar=1.0 / LO_SCALE,
                    in1=psA[:, :E],
                    op0=mybir.AluOpType.mult,
                    op1=mybir.AluOpType.add,
                )
                # softmax + top-6 (no max subtraction; |logits| < ~6)
                erow = small.tile([P, E], f32, tag="erow")
                sume = small.tile([P, 1], f32, tag="sume")
                nc.scalar.activation(erow, lg, EXP)
                nc.vector.tensor_reduce(
                    sume, erow, axis=mybir.AxisListType.X, op=mybir.AluOpType.add
                )
                v8 = small.tile([P, 8], f32, tag="v8")
                nc.vector.max(out=v8, in_=lg)
                i8 = small.tile([P, 8], u32, tag="i8")
                nc.vector.max_index(out=i8, in_max=v8, in_values=lg)
                rec = small.tile([P, 1], f32, tag="rec")
                nc.vector.reciprocal(rec, sume)
                ev = small.tile([P, TOPK], f32, tag="ev")
                nc.scalar.activation(ev, v8[:, :TOPK], EXP)
                nc.vector.tensor_scalar_mul(vstage[:, b], ev, rec)
                nc.vector.tensor_copy(istage[:, b], i8[:, :TOPK])

            nc.sync.dma_start(out=idx_o[:], in_=istage)
            nc.scalar.dma_start(out=val_o[:], in_=vstage)
    if not nc.is_finalized():
        nc.finalize()
    return nc


def _get_nc(t_shard: int) -> "bass.Bass":
    if t_shard not in _NC_CACHE:
        _NC_CACHE[t_shard] = build_nc(t_shard)
    return _NC_CACHE[t_shard]


def _split_hi_lo(a: np.ndarray) -> tuple[np.ndarray, np.ndarray]:
    hi = a.astype(np.float16)
    lo = ((a - hi.astype(np.float32)) * np.float32(LO_SCALE)).astype(np.float16)
    return hi, lo


def pack_core_input(xh: np.ndarray, xl: np.ndarray) -> np.ndarray:
    """[t_shard, K] hi/lo -> [P, nt*KC*2*P] packed for the kernel."""
    t_shard = xh.shape[0]
    nt = t_shard // P
    # [t, (c p)] -> [p, b(t-tile), c, h, t-in-tile]
    xp = np.empty((P, nt, KC, 2, P), np.float16)
    hiT = xh.T.reshape(KC, P, nt, P)  # [c, p, b, t]
    loT = xl.T.reshape(KC, P, nt, P)
    xp[:, :, :, 0, :] = hiT.transpose(1, 2, 0, 3)
    xp[:, :, :, 1, :] = loT.transpose(1, 2, 0, 3)
    return np.ascontiguousarray(xp.reshape(P, nt * KC * 2 * P))


def kernel(x: np.ndarray, W_g: np.ndarray, **run_kwargs):
    global LAST_RESULT
    x = np.asarray(x, dtype=np.float32)
    W = np.asarray(W_g, dtype=np.float32)
    t_shard = x.shape[0] // N_CORES
    nc = _get_nc(t_shard)

    xh, xl = _split_hi_lo(x)
    Wh, Wl = _split_hi_lo(W)
    Wstk = np.concatenate([Wh, Wl], axis=1)  # [K, 2E]
    # [(c p), e] -> [p, (c e)]
    Ws = np.ascontiguousarray(
        Wstk.reshape(KC, P, 2 * E).transpose(1, 0, 2).reshape(P, KC * 2 * E)
    )
    in_maps = [
        {
            "xhl": pack_core_input(
                xh[c * t_shard : (c + 1) * t_shard],
                xl[c * t_shard : (c + 1) * t_shard],
            ),
            "Ws": Ws,
        }
        for c in range(N_CORES)
    ]
    res = bass_utils.run_bass_kernel_spmd(
        nc, in_maps, core_ids=list(range(N_CORES)), **run_kwargs
    )
    LAST_RESULT = res
    # device layout is [P, nt, TOPK]; token t = tile*P + p -> [t_shard, TOPK]
    idx = np.concatenate(
        [np.moveaxis(r["idx"], 0, 1).reshape(t_shard, TOPK) for r in res.results],
        axis=0,
    ).astype(np.int32)
    val = np.concatenate(
        [np.moveaxis(r["val"], 0, 1).reshape(t_shard, TOPK) for r in res.results],
        axis=0,
    ).astype(np.float32)
    return idx, val
